# revision 24
# baseline (speedup 1.0000x reference)
"""DenseCaptioner LSTM-gate kernel for 8 Trainium2 NeuronCores.

Role-split sharding (zero weight replication -> minimal HBM traffic):
  cores 0-3  run program VIS: visual + recurrent paths for gate g = core,
             full batch -> partial logits [256,1024]
  cores 4-7  run program INP: input path for gate g = core-4, full batch
             -> partial logits [256,1024]
Host: logits[g] = vis_part[g] + inp_part[g] + b[g], then sigmoid/tanh gate
math and the prev_c recurrence.

All matmuls run in bf16 (tol 2e-2 gives ample margin; halves HBM traffic
and enables fast weight load). Layout avoids every transpose:
  level-1/2 matmuls are weight-stationary: lhsT = weight k-tile column
  block [128k,128h], rhs = activation^T image [128k,256b], producing the
  intermediate directly transposed as [h,b].  Hadamard products happen in
  [h,b] layout on DVE, and level-3 consumes the [h,b] image as the
  stationary operand with the output-level weights streaming, producing
  [b,H2] exactly as needed for the output DMA.

PSUM: each [128,512] fp32 tile is one bank holding TWO 256-wide
accumulation groups. A start=True matmul clears has_written for the whole
bank, so only the bank's FIRST matmul of a phase uses start=True; the
second group's first matmul relies on flags=0 overwrite-where-bit-clear.

Weights are pre-tiled on the host to the SBUF image layout so every
weight-chunk DMA is a fully contiguous per-partition copy. Activation
chunk DMAs are issued just-in-time between weight chunks so the PE never
waits behind queued activation bytes.
"""

import numpy as np

import jax
from jax.experimental.shard_map import shard_map
from jax.sharding import Mesh, PartitionSpec

import concourse.mybir as mybir
import concourse.tile as tile
from concourse import bacc, bass2jax

B, X, V, MM, VH, H1, H2, G = 256, 12000, 4096, 1024, 1024, 1024, 1024, 4
XP = 12032  # X padded to a multiple of 128 (94 k-tiles)
N_CORES = 8
HC = 8  # output column chunks of 128 per [*,1024] weight (h-chunks)

DT = mybir.dt.bfloat16
NPDT = mybir.dt.np(DT)

_cache = {}


def _big_chunks(nkt):
    """k-tile chunking for the long weight streams: tiny first chunks for
    a fast PE ramp, then 8-k-tile (2 MiB) chunks for DMA efficiency."""
    ck = [1, 2, 4]
    left = nkt - 7
    while left:
        c = min(8, left)
        ck.append(c)
        left -= c
    return ck


def _act_chunks(nkt):
    ck = [min(2, nkt)]
    left = nkt - ck[0]
    if left:
        c = min(6, left)
        ck.append(c)
        left -= c
    while left:
        c = min(8, left)
        ck.append(c)
        left -= c
    return ck


def build_program(role):
    """role "vis": visual+recurrent paths; "inp": input path. Full batch."""
    dt = DT
    f32 = mybir.dt.float32

    nc = bacc.Bacc("TRN2", target_bir_lowering=False, debug=False)

    if role == "vis":
        act_specs = {"v1T": V, "v2T": V, "mT": MM, "hT": H2}
        w_specs = {"V1": V, "V2": V, "C1": VH, "C2": MM, "C3": H1,
                   "U1": H2, "U2": MM, "U3": H1}
    else:
        act_specs = {"xT": XP, "mT": MM}
        w_specs = {"W1": XP, "W2": MM, "W3": H1}

    acts_d = {
        name: nc.dram_tensor(name, [128, k // 128 * B], dt, kind="ExternalInput")
        for name, k in act_specs.items()
    }
    # weights pre-tiled on host: [128, (K/128)*H1], image (p, t*H1+n)
    wt = {
        name: nc.dram_tensor(name, [128, k // 128 * H1], dt,
                             kind="ExternalInput")
        for name, k in w_specs.items()
    }
    out = nc.dram_tensor("out", [B, H2], dt, kind="ExternalOutput")

    with tile.TileContext(nc) as tc:
        with (
            tc.tile_pool(name="acts", bufs=1) as acts,
            tc.tile_pool(name="wbig", bufs=3 if role == "vis" else 5) as wbig,
            tc.tile_pool(name="wsmall", bufs=4 if role == "vis" else 2) as wsmall,
            tc.tile_pool(name="inter", bufs=1) as inter,
            tc.tile_pool(name="ps", bufs=2, space="PSUM") as ps,
        ):
            act_tiles = {
                name: acts.tile([128, (k // 128) * B], dt, tag=name,
                                name=f"act_{name}")
                for name, k in act_specs.items()
            }
            act_sb = {
                name: t.rearrange("p (t b) -> p t b", b=B)
                for name, t in act_tiles.items()
            }

            # just-in-time activation chunk loader
            act_state = {name: 0 for name in act_specs}  # k-tiles issued

            def act_dma(name, nkt_cap=None):
                """Issue the next pending chunk DMA for activation `name`.
                Returns False when fully issued."""
                nkt = act_specs[name] // 128
                done = act_state[name]
                if done >= nkt:
                    return False
                ck = _act_chunks(nkt)
                # find the chunk starting at `done`
                off = 0
                for c in ck:
                    if off == done:
                        sl = slice(off * B, (off + c) * B)
                        nc.sync.dma_start(
                            act_tiles[name][:, sl], acts_d[name].ap()[:, sl])
                        act_state[name] = off + c
                        return True
                    off += c
                return False

            # weight chunk queues: prefetch closures push (tile, off, ck)
            wq = {name: [] for name in w_specs}
            w_off = {name: 0 for name in w_specs}

            def w_issue(name, ck, pool):
                off = w_off[name]
                tag = "wb" if pool is wbig else "ws"
                t = pool.tile([128, ck * H1], dt, tag=tag,
                              name=f"w_{name}_{off}")
                nc.sync.dma_start(
                    t[:], wt[name].ap()[:, off * H1:(off + ck) * H1])
                wq[name].append((t, off, ck))
                w_off[name] = off + ck

            def pre_small(name):
                """Prefetch a whole 8-k-tile matrix through wsmall."""
                return lambda: w_issue(name, HC, wsmall)

            def mm8(pss, wav, tl, act, kt, nkt):
                """One k-tile's 8 h-chunk matmuls into pss."""
                for hc in range(HC):
                    nc.tensor.matmul(
                        pss[hc // 2][:, (hc % 2) * 256:(hc % 2 + 1) * 256],
                        wav[:, tl, hc * 128:(hc + 1) * 128],
                        act(kt),
                        start=(kt == 0 and hc % 2 == 0),
                        stop=(kt == nkt - 1),
                        skip_group_check=(hc % 2 == 1))

            def l1_single(w_name, act_fn, plan, pool, side=None,
                          merged=None):
                """pss[i] [128,512] = h-chunk pair (2i,2i+1) of (act @ W)^T
                in [h,b] layout; weight-stationary, weight k-tiles
                streaming; side = {chunk_idx: [closures]} issued ahead of
                that chunk's weight DMA (activation or weight prefetch).
                merged = (wb_name, act_b_fn): a second 8-k-tile stream
                whose matmuls are spread evenly through this loop (its
                weight chunks must be issued via side closures)."""
                nkt = w_specs[w_name] // 128
                assert sum(plan) == nkt
                pss = [ps.tile([128, 512], f32, tag=f"ps{i}",
                               name=f"ps_{w_name}{i}") for i in range(4)]
                psb = None
                if merged is not None:
                    wb_name, act_b = merged
                    nkt_b = w_specs[wb_name] // 128
                    stride = nkt // nkt_b
                    psb = [ps.tile([128, 512], f32, tag=f"ps{i}",
                                   name=f"psb_{wb_name}{i}") for i in range(4)]
                side = side or {}
                qpos = 0
                for ci, ck in enumerate(plan):
                    for fn in side.get(ci, ()):
                        fn()
                    if qpos >= len(wq[w_name]):
                        w_issue(w_name, ck, pool)
                    wa, off, cka = wq[w_name][qpos]
                    qpos += 1
                    wav = wa.rearrange("p (t n) -> p t n", t=cka)
                    for tl in range(cka):
                        kt = off + tl
                        mm8(pss, wav, tl, act_fn, kt, nkt)
                        if merged is not None and (kt + 1) % stride == 0:
                            j = (kt + 1) // stride - 1
                            wbt, boff, ckb = next(
                                e for e in wq[wb_name]
                                if e[1] <= j < e[1] + e[2])
                            wbv = wbt.rearrange("p (t n) -> p t n", t=ckb)
                            mm8(psb, wbv, j - boff, act_b, j, nkt_b)
                return (pss, psb) if merged is not None else pss

            def evac_bf16(pss, tag):
                """psum tiles -> one SBUF bf16 image [128, 4*512]."""
                t = inter.tile([128, HC * B], dt, tag=tag)
                for i in range(4):
                    nc.vector.tensor_copy(
                        t[:, i * 512:(i + 1) * 512], pss[i][:])
                return t

            def mul_q(pss, tmp, qname):
                """q[h,b] = psum * tmp -> 4 bf16 tiles [128,512]."""
                qts = []
                for i in range(4):
                    q = inter.tile([128, 512], dt, tag=f"{qname}{i}")
                    nc.vector.tensor_mul(
                        q[:], tmp[:, i * 512:(i + 1) * 512], pss[i][:])
                    qts.append(q)

                def q_rhs(kt):  # [128,256] rhs slice for k-tile kt
                    return qts[kt // 2][:, (kt % 2) * 256:(kt % 2 + 1) * 256]

                def q_lhs(kt, bc):  # [128,128] stationary slice
                    o = (kt % 2) * 256 + bc * 128
                    return qts[kt // 2][:, o:o + 128]

                return q_rhs, q_lhs

            def l3_acc(pacc, q_lhs, w_name, first, last, emit_out=None):
                """pacc[bc*2+nh] [128b,512] += q^T @ W (act-stationary,
                weights prefetched, contraction over the 8 h-tiles).
                With emit_out set (final matrix), the loop runs batch-chunk
                -major so each half's output copy/DMA overlaps the other
                half's matmuls."""
                if not wq[w_name]:
                    w_issue(w_name, HC, wsmall)
                w3, _, ck = wq[w_name][0]
                w3v = w3.rearrange("p (t n) -> p t n", t=ck)
                if emit_out is None:
                    for kt in range(HC):
                        for bc in range(2):
                            for nh in range(2):
                                nc.tensor.matmul(
                                    pacc[bc * 2 + nh][:],
                                    q_lhs(kt, bc),
                                    w3v[:, kt, nh * 512:(nh + 1) * 512],
                                    start=(first and kt == 0),
                                    stop=(last and kt == HC - 1))
                else:
                    for bc in range(2):
                        for kt in range(HC):
                            for nh in range(2):
                                nc.tensor.matmul(
                                    pacc[bc * 2 + nh][:],
                                    q_lhs(kt, bc),
                                    w3v[:, kt, nh * 512:(nh + 1) * 512],
                                    start=(first and kt == 0),
                                    stop=(last and kt == HC - 1))
                        emit_out(bc)

            outsb = inter.tile([128, 2 * H2], dt, tag="outsb")
            out_v = out.ap().rearrange("(m p) n -> m p n", p=128)

            def make_emit_out(pacc):
                def emit_out(bc):
                    for nh in range(2):
                        nc.vector.tensor_copy(
                            outsb[:, bc * H2 + nh * 512:
                                  bc * H2 + (nh + 1) * 512],
                            pacc[bc * 2 + nh][:])
                    nc.sync.dma_start(
                        out_v[bc], outsb[:, bc * H2:(bc + 1) * H2])
                return emit_out

            A = act_dma
            if role == "vis":
                # temporal DMA balancing: the independent 8-k-tile streams
                # (U1, U2, C2 — none depend on V outputs) are merged into
                # the long V1/V2/C1 loops so HBM demand stays flat.
                act_dma("v1T")
                psaV, psbU1 = l1_single(
                    "V1", lambda k: act_sb["v1T"][:, k, :],
                    _big_chunks(V // 128), wbig,
                    merged=("U1", lambda k: act_sb["hT"][:, k, :]),
                    side={0: [lambda: A("v1T"),
                              lambda: w_issue("U1", 2, wsmall)],
                          1: [lambda: A("hT")],
                          2: [lambda: A("v1T"),
                              lambda: w_issue("U1", 3, wsmall)],
                          3: [lambda: A("hT")],
                          4: [lambda: A("v1T"),
                              lambda: w_issue("U1", 3, wsmall)],
                          5: [lambda: A("v1T"), lambda: A("v2T")]})
                t1a = evac_bf16(psaV, "t1a")
                tua = evac_bf16(psbU1, "tua")
                psaV2, psbU2 = l1_single(
                    "V2", lambda k: act_sb["v2T"][:, k, :],
                    _big_chunks(V // 128), wbig,
                    merged=("U2", lambda k: act_sb["mT"][:, k, :]),
                    side={0: [lambda: A("v2T"),
                              lambda: w_issue("U2", 2, wsmall)],
                          1: [lambda: A("v2T"), lambda: A("mT")],
                          2: [lambda: A("v2T"), lambda: A("mT"),
                              lambda: w_issue("U2", 3, wsmall)],
                          3: [lambda: A("v2T")],
                          4: [lambda: w_issue("U2", 3, wsmall)],
                          5: [pre_small("C1")],
                          6: [pre_small("C2")]})
                q1_rhs, _ = mul_q(psaV2, t1a, "q1")
                _, qu_lhs = mul_q(psbU2, tua, "qu")
                # level-2: z^T=(t1@C1)^T, t2^T=(m@C2)^T, Hadamard
                psaC, psbC2 = l1_single(
                    "C1", q1_rhs, [HC], wsmall,
                    merged=("C2", lambda k: act_sb["mT"][:, k, :]),
                    side={0: [pre_small("U3"), pre_small("C3")]})
                tza = evac_bf16(psaC, "tza")
                _, q2_lhs = mul_q(psbC2, tza, "q2")
                # level-3: acc[b,H2] = (t5*t6)@U3 + (z*t2)@C3
                # (U3 first: qu is ready long before q2)
                pacc = [ps.tile([128, 512], f32, tag=f"ps{i}",
                                name=f"pacc{i}") for i in range(4)]
                l3_acc(pacc, qu_lhs, "U3", first=True, last=False)
                l3_acc(pacc, q2_lhs, "C3", first=False, last=True,
                       emit_out=make_emit_out(pacc))
            else:
                # W2 (8 k-tiles over m) merged into W1's k-loop: at kt<8
                # each iteration issues both paths' matmuls, so the PE
                # never waits on a W2->W1 phase boundary.
                act_dma("mT")
                w_issue("W2", 2, wsmall)
                psb = [ps.tile([128, 512], f32, tag=f"ps{i}",
                               name=f"psb_W2{i}") for i in range(4)]
                psa = [ps.tile([128, 512], f32, tag=f"ps{i}",
                               name=f"psa_W1{i}") for i in range(4)]
                plan = _big_chunks(XP // 128)
                nkt = XP // 128
                side = dict([(i, [lambda: A("xT")]) for i in range(3, 11)]
                            + [(11, [pre_small("W3")]), (12, [lambda: A("xT")])])
                side[0] = [lambda: A("xT")]
                side[1] = [lambda: A("xT"), lambda: A("mT")]
                side[2] = [lambda: A("xT"), lambda: A("xT")]
                t4a = None
                qpos = 0
                off = 0
                for ci, ck in enumerate(plan):
                    for fn in side.get(ci, ()):
                        fn()
                    if w_off["W2"] < min(8, off + ck + 2):
                        w_issue("W2", min(3, 8 - w_off["W2"]), wsmall)
                    if qpos >= len(wq["W1"]):
                        w_issue("W1", ck, wbig)
                    wa, aoff, cka = wq["W1"][qpos]
                    qpos += 1
                    wav = wa.rearrange("p (t n) -> p t n", t=cka)
                    for tl in range(cka):
                        kt = aoff + tl
                        if kt < 8:
                            wbt, boff, ckb = next(
                                e for e in wq["W2"]
                                if e[1] <= kt < e[1] + e[2])
                            wbv = wbt.rearrange("p (t n) -> p t n", t=ckb)
                            for hc in range(HC):
                                nc.tensor.matmul(
                                    psb[hc // 2][:, (hc % 2) * 256:(hc % 2 + 1) * 256],
                                    wbv[:, kt - boff, hc * 128:(hc + 1) * 128],
                                    act_sb["mT"][:, kt, :],
                                    start=(kt == 0 and hc % 2 == 0),
                                    stop=(kt == 7),
                                    skip_group_check=(hc % 2 == 1))
                        for hc in range(HC):
                            nc.tensor.matmul(
                                psa[hc // 2][:, (hc % 2) * 256:(hc % 2 + 1) * 256],
                                wav[:, tl, hc * 128:(hc + 1) * 128],
                                act_sb["xT"][:, kt, :],
                                start=(kt == 0 and hc % 2 == 0),
                                stop=(kt == nkt - 1),
                                skip_group_check=(hc % 2 == 1))
                        if kt == 7:
                            t4a = evac_bf16(psb, "t4a")
                    off += ck
                q3_rhs, q3_lhs = mul_q(psa, t4a, "q3")
                pacc = [ps.tile([128, 512], f32, tag=f"ps{i}",
                                name=f"pacc{i}") for i in range(4)]
                l3_acc(pacc, q3_lhs, "W3", first=True, last=True,
                       emit_out=make_emit_out(pacc))

            for name, k in act_specs.items():
                assert act_state[name] == k // 128, (
                    f"activation {name} not fully issued: "
                    f"{act_state[name]} of {k // 128} k-tiles")

    nc.compile()
    return nc


def _make_runner(nc, devices):
    """Adapted from concourse.bass2jax.run_bass_via_pjrt: same lowering,
    but runs on an explicit device subset and returns unmaterialized jax
    arrays so two programs can be dispatched concurrently."""
    bass2jax.install_neuronx_cc_hook()

    assert nc.dbg_addr is None
    partition_name = (
        nc.partition_id_tensor.name if nc.partition_id_tensor else None
    )

    in_names, out_names, out_avals, zero_outs = [], [], [], []
    for alloc in nc.m.functions[0].allocations:
        if not isinstance(alloc, mybir.MemoryLocationSet):
            continue
        name = alloc.memorylocations[0].name
        if alloc.kind == "ExternalInput":
            if name != partition_name:
                in_names.append(name)
        elif alloc.kind == "ExternalOutput":
            shape = tuple(alloc.tensor_shape)
            dtype = mybir.dt.np(alloc.dtype)
            out_names.append(name)
            out_avals.append(jax.core.ShapedArray(shape, dtype))
            zero_outs.append(np.zeros(shape, dtype))
    n_params = len(in_names)
    n_outs = len(out_avals)
    in_names.extend(out_names)
    if partition_name is not None:
        in_names.append(partition_name)
    donate = tuple(range(n_params, n_params + n_outs))

    def _body(*args):
        operands = list(args)
        if partition_name is not None:
            operands.append(bass2jax.partition_id_tensor())
        outs = bass2jax._bass_exec_p.bind(
            *operands,
            out_avals=tuple(out_avals),
            in_names=tuple(in_names),
            out_names=tuple(out_names),
            lowering_input_output_aliases=(),
            sim_require_finite=True,
            sim_require_nnan=True,
            nc=nc,
        )
        return tuple(outs)

    n_cores = len(devices)
    mesh = Mesh(np.asarray(devices), ("core",))
    in_specs = (PartitionSpec("core"),) * (n_params + n_outs)
    out_specs = (PartitionSpec("core"),) * n_outs
    sharded = jax.jit(
        shard_map(
            _body, mesh=mesh, in_specs=in_specs, out_specs=out_specs,
            check_rep=False,
        ),
        donate_argnums=donate,
        keep_unused=True,
    )

    def run(in_maps):
        assert len(in_maps) == n_cores
        concat_in = [
            np.concatenate(
                [np.asarray(in_maps[c][name]) for c in range(n_cores)], axis=0
            )
            for name in in_names[:n_params]
        ]
        concat_zeros = [
            np.zeros((n_cores * z.shape[0], *z.shape[1:]), z.dtype)
            for z in zero_outs
        ]
        out_arrs = sharded(*concat_in, *concat_zeros)
        return out_names, out_avals, out_arrs

    return run


def _tile_actT(a, kdim):
    """[256 batch, K<=kdim] -> SBUF image [128, (kdim/128) * 256]:
    (p, t*256+b) = a[b, t*128+p] — partition = k within tile, col = batch."""
    ktiles = kdim // 128
    a = np.asarray(a, np.float32)
    if a.shape[1] < kdim:
        a = np.pad(a, ((0, 0), (0, kdim - a.shape[1])))
    r = a.reshape(B, ktiles, 128).transpose(2, 1, 0)
    return np.ascontiguousarray(r.reshape(128, ktiles * B)).astype(NPDT)


def _tile_w(w, kdim):
    """[K, H1] -> [128, (K/128)*H1] image: (p, t*H1+n) = w[t*128+p, n]."""
    kt = kdim // 128
    w = np.asarray(w, np.float32)
    if w.shape[0] < kdim:
        w = np.pad(w, ((0, kdim - w.shape[0]), (0, 0)))
    r = w.reshape(kt, 128, H1).transpose(1, 0, 2)
    return np.ascontiguousarray(r.reshape(128, kt * H1)).astype(NPDT)


def kernel(prev_h, prev_c, x, m, v1, v2, V1, V2, C1, C2, C3, W1, W2, W3, U1, U2, U3, b):
    if "runners" not in _cache:
        devs = jax.devices()
        nc_vis = build_program("vis")
        nc_inp = build_program("inp")
        _cache["runners"] = (
            _make_runner(nc_vis, devs[0:4]),
            _make_runner(nc_inp, devs[4:8]),
        )
        _cache["ncs"] = (nc_vis, nc_inp)
    run_vis, run_inp = _cache["runners"]

    v1T_img = _tile_actT(v1, V)
    v2T_img = _tile_actT(v2, V)
    mT_img = _tile_actT(m, MM)
    hT_img = _tile_actT(prev_h, H2)
    xT_img = _tile_actT(x, XP)

    vis_maps, inp_maps = [], []
    for g in range(G):
        vis_maps.append({
            "v1T": v1T_img, "v2T": v2T_img, "mT": mT_img, "hT": hT_img,
            "V1": _tile_w(V1[g], V), "V2": _tile_w(V2[g], V),
            "C1": _tile_w(C1[g], VH), "C2": _tile_w(C2[g], MM),
            "C3": _tile_w(C3[g], H1), "U1": _tile_w(U1[g], H2),
            "U2": _tile_w(U2[g], MM), "U3": _tile_w(U3[g], H1),
        })
        inp_maps.append({
            "xT": xT_img, "mT": mT_img,
            "W1": _tile_w(W1[g], XP), "W2": _tile_w(W2[g], MM),
            "W3": _tile_w(W3[g], H1),
        })

    _cache["last_in_maps"] = (vis_maps, inp_maps)

    # dispatch both programs; they run concurrently on disjoint cores
    vnames, vavals, vouts = run_vis(vis_maps)
    inames, iavals, iouts = run_inp(inp_maps)

    vis_out = np.asarray(vouts[0]).astype(np.float32).reshape(G, B, H2)
    inp_out = np.asarray(iouts[0]).astype(np.float32).reshape(G, B, H2)

    logits = vis_out + inp_out + np.asarray(b, np.float32)[:, None, :]

    def sigmoid(z):
        return 1.0 / (1.0 + np.exp(-z))

    i = sigmoid(logits[0])
    f = sigmoid(logits[1])
    o = sigmoid(logits[2])
    cg = np.tanh(logits[3])
    prev_c = np.asarray(prev_c, np.float32)
    new_c = f * prev_c + i * cg
    new_h = o * np.tanh(prev_c)
    return new_h.astype(np.float32), new_c.astype(np.float32)
